# revision 1
# baseline (speedup 1.0000x reference)
import numpy as np

from concourse import bass, bacc, mybir, tile
from concourse.bass_utils import run_bass_kernel_spmd

F32 = mybir.dt.float32
I32 = mybir.dt.int32

T, R, D, H, DK, L = 3, 6, 128, 4, 32, 2
REL_SRC = (0, 1, 2, 0, 1, 2)
REL_DST = (1, 2, 0, 2, 0, 1)
SQRT_DK = float(np.sqrt(DK))
EPS = 1e-5
NCORE = 8
CAP = 256
RELS_OF = [[r for r in range(R) if REL_DST[r] == t] for t in range(T)]


# ---------------- host-side packing ----------------

def pack(names, src_idx, dst_idx, N):
    ntile = (N + NCORE * 128 - 1) // (NCORE * 128)
    nslot = ntile * 128
    nch = 2 * ntile
    deg = np.stack([np.bincount(dst_idx[r], minlength=N) for r in range(R)])
    owner = np.zeros((T, N), np.int32)
    slot = np.zeros((T, N), np.int32)
    NB = NCORE * ntile
    for t in range(T):
        r1, r2 = RELS_OF[t]
        order = np.argsort(-(deg[r1] + deg[r2]), kind='stable')
        bins = [[] for _ in range(NB)]
        load1 = np.zeros(NB, np.int64)
        load2 = np.zeros(NB, np.int64)
        for k in range(0, N, NB):
            nodes = order[k:k + NB]
            seq = range(NB) if (k // NB) % 2 == 0 else range(NB - 1, -1, -1)
            for n, b in zip(nodes, seq):
                bins[b].append(n)
                load1[b] += deg[r1][n]
                load2[b] += deg[r2][n]
        sizes = np.array([len(b) for b in bins])
        for _ in range(400):
            bad = np.where((load1 > CAP) | (load2 > CAP))[0]
            if len(bad) == 0:
                break
            for b in bad:
                while load1[b] > CAP or load2[b] > CAP:
                    nb = max(bins[b], key=lambda n: deg[r1][n] + deg[r2][n])
                    cand = int(np.argmin(load1 + load2 + (sizes >= 128) * (1 << 40)))
                    bins[b].remove(nb)
                    load1[b] -= deg[r1][nb]; load2[b] -= deg[r2][nb]; sizes[b] -= 1
                    bins[cand].append(nb)
                    load1[cand] += deg[r1][nb]; load2[cand] += deg[r2][nb]; sizes[cand] += 1
        assert (load1 <= CAP).all() and (load2 <= CAP).all()
        for b in range(NB):
            c, tl = b % NCORE, b // NCORE
            for p, n in enumerate(bins[b]):
                owner[t][n] = c
                slot[t][n] = tl * 128 + p

    rowid = owner.astype(np.int64) * (T * nslot) + \
        (np.arange(T)[:, None] * nslot) + slot  # [T,N]

    node_at = np.full((T, NCORE, nslot), -1, np.int64)
    for t in range(T):
        node_at[t, owner[t], slot[t]] = np.arange(N)

    ECH = nch * 128
    srcrowT = np.zeros((R, NCORE, 128, nch), np.int32)
    qtidxT = np.zeros((R, NCORE, 128, nch), np.int32)
    dstoffT = np.full((R, NCORE, 128, nch), 200.0, np.float32)
    for r in range(R):
        st, dt = REL_SRC[r], REL_DST[r]
        s, d = src_idx[r], dst_idx[r]
        ce = owner[dt][d]
        sl = slot[dt][d]
        srow = rowid[st][s].astype(np.int32)
        for c in range(NCORE):
            m = ce == c
            tl = (sl[m] >> 7).astype(np.int64)
            o2 = np.argsort(tl, kind='stable')
            tls = tl[o2]
            cnt = np.bincount(tls, minlength=ntile)
            starts = np.zeros(ntile, np.int64)
            starts[1:] = np.cumsum(cnt)[:-1]
            within = np.arange(len(tls)) - np.repeat(starts, cnt)
            place = tls * CAP + within
            SR = np.zeros(ECH, np.int32)
            QI = np.zeros(ECH, np.int32)
            DO = np.full(ECH, 200.0, np.float32)
            SR[place] = srow[m][o2]
            QI[place] = sl[m][o2]
            DO[place] = (sl[m] & 127)[o2].astype(np.float32)
            srcrowT[r, c] = SR.reshape(nch, 128).T
            qtidxT[r, c] = QI.reshape(nch, 128).T
            dstoffT[r, c] = DO.reshape(nch, 128).T

    cntn = np.zeros((T, N), np.float32)
    for t in range(T):
        for r in RELS_OF[t]:
            cntn[t] += (deg[r] > 0)
    invn = 1.0 / np.maximum(cntn, 1.0)
    invT = np.ones((NCORE, T, 128, ntile), np.float32)
    embidxT = np.zeros((NCORE, 128, T * ntile), np.int32)
    for t in range(T):
        for c in range(NCORE):
            na = node_at[t, c]
            live = na >= 0
            iv = np.ones(nslot, np.float32)
            iv[live] = invn[t][na[live]]
            invT[c, t] = iv.reshape(ntile, 128).T
            er = np.zeros(nslot, np.int32)
            er[live] = names[t][na[live]]
            embidxT[c, :, t * ntile:(t + 1) * ntile] = er.reshape(ntile, 128).T

    return dict(ntile=ntile, nslot=nslot, nch=nch, owner=owner, slot=slot,
                srcrowT=srcrowT, qtidxT=qtidxT, dstoffT=dstoffT,
                invT=invT, embidxT=embidxT, node_at=node_at)


def fold_weights(w):
    KW = np.zeros((L, T, D, D), np.float32)
    WMSG = np.zeros((L, R, D, D), np.float32)
    MSGB = np.zeros((L, R, D), np.float32)
    W2 = np.zeros((L, R, D, 132), np.float32)
    B2 = np.zeros((L, R, 132), np.float32)
    for l in range(L):
        for t in range(T):
            KW[l, t] = w['k_w'][l, t]
        for r in range(R):
            st, dt = REL_SRC[r], REL_DST[r]
            ratp = w['rel_att'][l, r] * (w['rel_pri'][l, r] / SQRT_DK)[:, None, None]
            M = np.zeros((D, D), np.float32)
            BD = np.zeros((D, D), np.float32)
            for h in range(H):
                M[h * DK:(h + 1) * DK, h * DK:(h + 1) * DK] = ratp[h].T
                BD[h * DK:(h + 1) * DK, h * DK:(h + 1) * DK] = w['rel_msg'][l, r, h]
            kb = w['k_b'][l, st]
            Ckb = np.zeros((D, H), np.float32)
            for h in range(H):
                Ckb[h * DK:(h + 1) * DK, h] = kb[h * DK:(h + 1) * DK]
            MA = np.concatenate([M, M @ Ckb], axis=1)  # [128,132]
            W2[l, r] = w['q_w'][l, dt] @ MA
            B2[l, r] = w['q_b'][l, dt] @ MA
            WMSG[l, r] = w['v_w'][l, st] @ BD
            MSGB[l, r] = w['v_b'][l, st] @ BD
    alphas = 1.0 / (1.0 + np.exp(-w['skip']))  # [L,T]
    return dict(KW=KW, WMSG=WMSG, MSGB=MSGB, W2=W2, B2=B2, alphas=alphas)


# ---------------- numpy model (validation mirror of the device program) ----

def numpy_forward(P, fw, emb, w, N):
    ntile, nslot, nch = P['ntile'], P['nslot'], P['nch']
    ROWS = NCORE * T * nslot
    # adapt
    xloc = []
    for c in range(NCORE):
        rows = P['embidxT'][c].T.reshape(-1)  # [T*nslot] local row-major
        inp = emb[rows]  # [T*nslot,128]
        h0 = np.zeros((T * nslot, D), np.float32)
        for t in range(T):
            blk = inp[t * nslot:(t + 1) * nslot]
            h0[t * nslot:(t + 1) * nslot] = np.tanh(blk @ w['adapt_w'][t] + w['adapt_b'][t])
        xloc.append(h0)
    xfull = np.concatenate(xloc, 0)
    assert xfull.shape == (ROWS, D)
    for l in range(L):
        newloc = []
        for c in range(NCORE):
            out_c = np.zeros((T * nslot, D), np.float32)
            for dt in range(T):
                tacc = np.zeros((nslot, D), np.float32)
                for r in RELS_OF[dt]:
                    st = REL_SRC[r]
                    xl = xloc[c][dt * nslot:(dt + 1) * nslot]
                    qt = xl @ fw['W2'][l, r] + fw['B2'][l, r]  # [nslot,132]
                    sr = P['srcrowT'][r, c].T.reshape(-1)  # [ECH]
                    qi = P['qtidxT'][r, c].T.reshape(-1)
                    do = P['dstoffT'][r, c].T.reshape(-1)
                    X = xfull[sr]  # [ECH,128]
                    ke = X @ fw['KW'][l, st]
                    QT = qt[qi]
                    att = (ke * QT[:, :D]).reshape(-1, H, DK).sum(-1) + QT[:, D:]
                    A = np.exp(att)  # [ECH,H]
                    msg = X @ fw['WMSG'][l, r]
                    mw = msg * np.repeat(A, DK, axis=1)
                    S = np.zeros((nslot, D), np.float32)
                    ss = np.zeros((nslot, H), np.float32)
                    for tl in range(ntile):
                        sl_ = slice(tl * CAP, (tl + 1) * CAP)
                        mask = do[sl_, None] == np.arange(128)[None, :]  # [CAP,128]
                        S[tl * 128:(tl + 1) * 128] += mask.T @ mw[sl_]
                        ss[tl * 128:(tl + 1) * 128] += mask.T @ A[sl_]
                    rec = 1.0 / (ss + 1e-20)
                    ind = ss * rec
                    hr = S * np.repeat(rec, DK, 1) + \
                        np.repeat(ind, DK, 1) * fw['MSGB'][l, r][None, :]
                    tacc += hr
                iv = P['invT'][c, dt].T.reshape(-1)
                tt = tacc * iv[:, None]
                trans = tt @ w['a_w'][l, dt] + w['a_b'][l, dt]
                al = fw['alphas'][l, dt]
                o = trans * al + xloc[c][dt * nslot:(dt + 1) * nslot] * (1 - al)
                mu = o.mean(-1, keepdims=True)
                var = ((o - mu) ** 2).mean(-1, keepdims=True)
                o = w['ln_g'][l, dt] * (o - mu) / np.sqrt(var + EPS) + w['ln_b'][l, dt]
                out_c[dt * nslot:(dt + 1) * nslot] = o
            newloc.append(out_c)
        xloc = newloc
        xfull = np.concatenate(xloc, 0)
    return xloc  # per-core local outputs [T*nslot, D]


def unpack_output(P, outs, N):
    nslot = P['nslot']
    res = np.zeros((T, N, D), np.float32)
    for t in range(T):
        ow, sl = P['owner'][t], P['slot'][t]
        allc = np.stack([outs[c][t * nslot:(t + 1) * nslot] for c in range(NCORE)])
        res[t] = allc[ow, sl]
    return res


# ---------------- device program ----------------

def build_nc(P, alphas):
    ntile, nslot, nch = P['ntile'], P['nslot'], P['nch']
    ROWS = NCORE * T * nslot
    nc = bacc.Bacc("TRN2", target_bir_lowering=False, debug=False, num_devices=NCORE)

    def din(name, shape, dt=F32):
        return nc.dram_tensor(name, list(shape), dt, kind="ExternalInput")

    emb_t = din("emb", (P['V'], D))
    embidx_t = din("embidx", (128, T * ntile), I32)
    srcrow_t = din("srcrow", (R * 128, nch), I32)
    qtidx_t = din("qtidx", (R * 128, nch), I32)
    dstoff_t = din("dstoff", (R * 128, nch))
    inv_t = din("invt", (T * 128, ntile))
    KW_t = din("kw", (L * T * 128, D))
    WMSG_t = din("wmsg", (L * R * 128, D))
    MSGB_t = din("msgb", (L * R * 128, D))
    W2_t = din("w2", (L * R * 128, 132))
    B2_t = din("b2", (L * R, 132))
    ADW_t = din("adw", (T * 128, D))
    ADB_t = din("adb", (T, D))
    AW_t = din("aw", (L * T * 128, D))
    AB_t = din("ab", (L * T, D))
    G_t = din("lng", (L * T * 128, D))
    BB_t = din("lnb", (L * T * 128, D))
    IOTA_t = din("iota", (128, 128))
    IDENT_t = din("ident", (128, 128))
    ONES_t = din("ones", (1, 128))
    out_t = nc.dram_tensor("outloc", [T * nslot, D], F32, kind="ExternalOutput")

    hloc = [nc.dram_tensor(f"hloc{l}", [T * nslot, D], F32) for l in range(L)]
    xfull = [nc.dram_tensor(f"xfull{l}", [ROWS, D], F32, addr_space="Shared")
             for l in range(L)]
    qtt = [nc.dram_tensor(f"qtt{i}", [nslot, 132], F32) for i in range(2)]

    from contextlib import ExitStack
    with tile.TileContext(nc) as tc, ExitStack() as es:
        cp = es.enter_context(tc.tile_pool(name="consts", bufs=1))
        ident = cp.tile([128, 128], F32); nc.sync.dma_start(out=ident[:], in_=IDENT_t[:, :])
        iota = cp.tile([128, 128], F32); nc.sync.dma_start(out=iota[:], in_=IOTA_t[:, :])
        ones = cp.tile([1, 128], F32); nc.sync.dma_start(out=ones[:], in_=ONES_t[:, :])
        epst = cp.tile([128, 1], F32); nc.vector.memset(epst[:], EPS)

        wp = es.enter_context(tc.tile_pool(name="wts", bufs=2))
        ip = es.enter_context(tc.tile_pool(name="idx", bufs=2))
        gp = es.enter_context(tc.tile_pool(name="gath", bufs=3))
        pp = es.enter_context(tc.tile_pool(name="ps", bufs=1, space="PSUM"))
        sp = es.enter_context(tc.tile_pool(name="work", bufs=3))
        ap_ = es.enter_context(tc.tile_pool(name="acc", bufs=1))

        def transpose_to(xt_ps_slice, src_ap):
            nc.tensor.transpose(out=xt_ps_slice, in_=src_ap, identity=ident[:])

        # ---- adapt phase ----
        embidx = ip.tile([128, T * ntile], I32, name="embidx_s")
        nc.sync.dma_start(out=embidx[:], in_=embidx_t[:, :])
        for t in range(T):
            adw = wp.tile([128, D], F32, tag="adw")
            nc.sync.dma_start(out=adw[:], in_=ADW_t[t * 128:(t + 1) * 128, :])
            adb = wp.tile([1, D], F32, tag="adb")
            nc.sync.dma_start(out=adb[:], in_=ADB_t[t:t + 1, :])
            for ch in range(ntile):
                g = gp.tile([128, D], F32, tag="eg")
                nc.gpsimd.indirect_dma_start(
                    out=g[:], out_offset=None, in_=emb_t[:, :],
                    in_offset=bass.IndirectOffsetOnAxis(
                        ap=embidx[:, t * ntile + ch:t * ntile + ch + 1], axis=0))
                tp = pp.tile([128, 128], F32, tag="tps")
                transpose_to(tp[:], g[:])
                gT = sp.tile([128, 128], F32, tag="gT")
                nc.vector.tensor_copy(out=gT[:], in_=tp[:])
                h0 = pp.tile([128, D], F32, tag="mm")
                nc.tensor.matmul(out=h0[:], lhsT=gT[:], rhs=adw[:], start=True, stop=False)
                nc.tensor.matmul(out=h0[:], lhsT=ones[:], rhs=adb[:], start=False, stop=True)
                hs = sp.tile([128, D], F32, tag="hs")
                nc.scalar.activation(out=hs[:], in_=h0[:],
                                     func=mybir.ActivationFunctionType.Tanh)
                base = t * nslot + ch * 128
                nc.sync.dma_start(out=hloc[0][base:base + 128, :], in_=hs[:])

        nc.gpsimd.collective_compute(
            "AllGather", mybir.AluOpType.bypass,
            replica_groups=[list(range(NCORE))],
            ins=[hloc[0].ap().opt()], outs=[xfull[0].ap().opt()])

        for l in range(L):
            for dt in range(T):
                # ---- qt phase ----
                w2s, b2s = [], []
                for ri, r in enumerate(RELS_OF[dt]):
                    w2 = wp.tile([128, 132], F32, tag=f"w2{ri}")
                    nc.sync.dma_start(out=w2[:], in_=W2_t[(l * R + r) * 128:(l * R + r + 1) * 128, :])
                    b2 = wp.tile([1, 132], F32, tag=f"b2{ri}")
                    nc.sync.dma_start(out=b2[:], in_=B2_t[l * R + r:l * R + r + 1, :])
                    w2s.append(w2); b2s.append(b2)
                for ch in range(ntile):
                    base = dt * nslot + ch * 128
                    xc = sp.tile([128, D], F32, tag="xc")
                    nc.sync.dma_start(out=xc[:], in_=hloc[l][base:base + 128, :])
                    tp = pp.tile([128, 128], F32, tag="tps")
                    transpose_to(tp[:], xc[:])
                    xcT = sp.tile([128, 128], F32, tag="xcT")
                    nc.vector.tensor_copy(out=xcT[:], in_=tp[:])
                    for ri in range(2):
                        qts = pp.tile([128, 132], F32, tag="qtp")
                        nc.tensor.matmul(out=qts[:], lhsT=xcT[:], rhs=w2s[ri][:], start=True, stop=False)
                        nc.tensor.matmul(out=qts[:], lhsT=ones[:], rhs=b2s[ri][:], start=False, stop=True)
                        qs = sp.tile([128, 132], F32, tag="qs")
                        nc.vector.tensor_copy(out=qs[:], in_=qts[:])
                        nc.sync.dma_start(out=qtt[ri][ch * 128:(ch + 1) * 128, :], in_=qs[:])

                tacc = ap_.tile([128, ntile * 128], F32, tag="tacc")
                ivt = ip.tile([128, ntile], F32, tag="ivt")
                nc.sync.dma_start(out=ivt[:], in_=inv_t[dt * 128:(dt + 1) * 128, :])

                # ---- edge phase ----
                for ri, r in enumerate(RELS_OF[dt]):
                    st = REL_SRC[r]
                    kw = wp.tile([128, D], F32, tag="kw")
                    nc.sync.dma_start(out=kw[:], in_=KW_t[(l * T + st) * 128:(l * T + st + 1) * 128, :])
                    wm = wp.tile([128, D], F32, tag="wm")
                    nc.sync.dma_start(out=wm[:], in_=WMSG_t[(l * R + r) * 128:(l * R + r + 1) * 128, :])
                    mbrep = wp.tile([128, D], F32, tag="mb")
                    nc.sync.dma_start(out=mbrep[:], in_=MSGB_t[(l * R + r) * 128:(l * R + r + 1) * 128, :])
                    srct = ip.tile([128, nch], I32, tag="srct")
                    nc.sync.dma_start(out=srct[:], in_=srcrow_t[r * 128:(r + 1) * 128, :])
                    qit = ip.tile([128, nch], I32, tag="qit")
                    nc.sync.dma_start(out=qit[:], in_=qtidx_t[r * 128:(r + 1) * 128, :])
                    dot = ip.tile([128, nch], F32, tag="dot")
                    nc.sync.dma_start(out=dot[:], in_=dstoff_t[r * 128:(r + 1) * 128, :])
                    for q in range(nch // 4):
                        XG = gp.tile([128, 4, 128], F32, tag="XG")
                        QT4 = gp.tile([128, 4, 132], F32, tag="QT4")
                        for c in range(4):
                            ch = q * 4 + c
                            nc.gpsimd.indirect_dma_start(
                                out=XG[:, c, :], out_offset=None, in_=xfull[l][:, :],
                                in_offset=bass.IndirectOffsetOnAxis(
                                    ap=srct[:, ch:ch + 1], axis=0))
                            nc.gpsimd.indirect_dma_start(
                                out=QT4[:, c, :], out_offset=None, in_=qtt[ri][:, :],
                                in_offset=bass.IndirectOffsetOnAxis(
                                    ap=qit[:, ch:ch + 1], axis=0))
                        tp4 = pp.tile([128, 512], F32, tag="tp4")
                        for c in range(4):
                            transpose_to(tp4[:, c * 128:(c + 1) * 128], XG[:, c, :])
                        XT4 = sp.tile([128, 512], F32, tag="XT4")
                        nc.vector.tensor_copy(out=XT4[:], in_=tp4[:])
                        keps = pp.tile([128, 4, 128], F32, tag="keps")
                        msps = pp.tile([128, 4, 128], F32, tag="msps")
                        for c in range(4):
                            nc.tensor.matmul(out=keps[:, c, :], lhsT=XT4[:, c * 128:(c + 1) * 128],
                                             rhs=kw[:], start=True, stop=True)
                            nc.tensor.matmul(out=msps[:, c, :], lhsT=XT4[:, c * 128:(c + 1) * 128],
                                             rhs=wm[:], start=True, stop=True)
                        P4 = sp.tile([128, 16, 32], F32, tag="P4")
                        nc.vector.tensor_tensor(out=P4[:], in0=keps[:], in1=QT4[:, :, 0:128],
                                                op=mybir.AluOpType.mult)
                        attE = sp.tile([128, 16], F32, tag="attE")
                        nc.vector.tensor_reduce(out=attE[:], in_=P4[:],
                                                axis=mybir.AxisListType.X,
                                                op=mybir.AluOpType.add)
                        nc.vector.tensor_tensor(out=attE[:], in0=attE[:], in1=QT4[:, :, 128:132],
                                                op=mybir.AluOpType.add)
                        A4 = sp.tile([128, 16, 1], F32, tag="A4")
                        nc.scalar.activation(out=A4[:], in_=attE[:],
                                             func=mybir.ActivationFunctionType.Exp)
                        mw4 = sp.tile([128, 4, 132], F32, tag="mw4")
                        nc.vector.tensor_tensor(out=mw4[:, :, 0:128], in0=msps[:],
                                                in1=A4[:].to_broadcast([128, 16, 32]),
                                                op=mybir.AluOpType.mult)
                        nc.vector.tensor_copy(out=mw4[:, :, 128:132], in_=A4[:])
                        for half in range(2):
                            Sps = pp.tile([128, 132], F32, tag="Sps")
                            for c2 in range(2):
                                c = half * 2 + c2
                                ch = q * 4 + c
                                msk = sp.tile([128, 128], F32, tag="msk")
                                nc.vector.tensor_tensor(
                                    out=msk[:], in0=dot[:, ch:ch + 1].to_broadcast([128, 128]),
                                    in1=iota[:], op=mybir.AluOpType.is_equal)
                                nc.tensor.matmul(out=Sps[:], lhsT=msk[:], rhs=mw4[:, c, :],
                                                 start=(c2 == 0), stop=(c2 == 1),
                                                 skip_group_check=True)
                            tl = q * 2 + half
                            rec = sp.tile([128, 4, 1], F32, tag="rec")
                            nc.vector.tensor_scalar(out=rec[:], in0=Sps[:, 128:132],
                                                    scalar1=1e-20, scalar2=None,
                                                    op0=mybir.AluOpType.add)
                            nc.vector.reciprocal(out=rec[:], in_=rec[:])
                            ind = sp.tile([128, 4, 1], F32, tag="ind")
                            nc.vector.tensor_tensor(out=ind[:], in0=Sps[:, 128:132],
                                                    in1=rec[:], op=mybir.AluOpType.mult)
                            hrA = sp.tile([128, 4, 32], F32, tag="hrA")
                            nc.vector.tensor_tensor(out=hrA[:], in0=Sps[:, 0:128],
                                                    in1=rec[:].to_broadcast([128, 4, 32]),
                                                    op=mybir.AluOpType.mult)
                            hrB = sp.tile([128, 4, 32], F32, tag="hrB")
                            # msgb replicated per partition via ones-broadcast matmul is
                            # avoided: use scalar engine copy with per-partition scale=ind
                            # on a row-replicated msgb tile built once per relation.
                            nc.vector.tensor_tensor(out=hrB[:], in0=mbrep[:],
                                                    in1=ind[:].to_broadcast([128, 4, 32]),
                                                    op=mybir.AluOpType.mult)
                            dst = tacc[:, tl * 128:(tl + 1) * 128]
                            if ri == 0:
                                nc.vector.tensor_tensor(out=dst, in0=hrA[:], in1=hrB[:],
                                                        op=mybir.AluOpType.add)
                            else:
                                nc.vector.tensor_tensor(out=dst, in0=dst, in1=hrA[:],
                                                        op=mybir.AluOpType.add)
                                nc.vector.tensor_tensor(out=dst, in0=dst, in1=hrB[:],
                                                        op=mybir.AluOpType.add)

                # ---- finish phase ----
                aw = wp.tile([128, D], F32, tag="aw")
                nc.sync.dma_start(out=aw[:], in_=AW_t[(l * T + dt) * 128:(l * T + dt + 1) * 128, :])
                ab = wp.tile([1, D], F32, tag="abb")
                nc.sync.dma_start(out=ab[:], in_=AB_t[l * T + dt:l * T + dt + 1, :])
                gt = wp.tile([128, D], F32, tag="gt")
                nc.sync.dma_start(out=gt[:], in_=G_t[(l * T + dt) * 128:(l * T + dt + 1) * 128, :])
                bt = wp.tile([128, D], F32, tag="bt")
                nc.sync.dma_start(out=bt[:], in_=BB_t[(l * T + dt) * 128:(l * T + dt + 1) * 128, :])
                al = float(alphas[l, dt])
                dest = hloc[1] if l == 0 else out_t
                for tl in range(ntile):
                    base = dt * nslot + tl * 128
                    tt = sp.tile([128, 128], F32, tag="tt")
                    nc.vector.tensor_scalar(out=tt[:], in0=tacc[:, tl * 128:(tl + 1) * 128],
                                            scalar1=ivt[:, tl:tl + 1], scalar2=None,
                                            op0=mybir.AluOpType.mult)
                    tp = pp.tile([128, 128], F32, tag="tps")
                    transpose_to(tp[:], tt[:])
                    ttT = sp.tile([128, 128], F32, tag="ttT")
                    nc.vector.tensor_copy(out=ttT[:], in_=tp[:])
                    trp = pp.tile([128, D], F32, tag="mm")
                    nc.tensor.matmul(out=trp[:], lhsT=ttT[:], rhs=aw[:], start=True, stop=False)
                    nc.tensor.matmul(out=trp[:], lhsT=ones[:], rhs=ab[:], start=False, stop=True)
                    o1 = sp.tile([128, D], F32, tag="o1")
                    nc.scalar.activation(out=o1[:], in_=trp[:],
                                         func=mybir.ActivationFunctionType.Copy, scale=al)
                    xc = sp.tile([128, D], F32, tag="xc2")
                    nc.sync.dma_start(out=xc[:], in_=hloc[l][base:base + 128, :])
                    o2 = sp.tile([128, D], F32, tag="o2")
                    nc.scalar.activation(out=o2[:], in_=xc[:],
                                         func=mybir.ActivationFunctionType.Copy, scale=1.0 - al)
                    nc.vector.tensor_tensor(out=o1[:], in0=o1[:], in1=o2[:],
                                            op=mybir.AluOpType.add)
                    mu = sp.tile([128, 1], F32, tag="mu")
                    nc.vector.tensor_reduce(out=mu[:], in_=o1[:], axis=mybir.AxisListType.X,
                                            op=mybir.AluOpType.add)
                    nc.scalar.activation(out=mu[:], in_=mu[:],
                                         func=mybir.ActivationFunctionType.Copy, scale=1.0 / 128)
                    xcn = sp.tile([128, D], F32, tag="xcn")
                    nc.vector.tensor_scalar(out=xcn[:], in0=o1[:], scalar1=mu[:, 0:1],
                                            scalar2=None, op0=mybir.AluOpType.subtract)
                    sq = sp.tile([128, D], F32, tag="sq")
                    vs = sp.tile([128, 1], F32, tag="vs")
                    nc.scalar.activation(out=sq[:], in_=xcn[:],
                                         func=mybir.ActivationFunctionType.Square,
                                         accum_out=vs[:])
                    nc.scalar.activation(out=vs[:], in_=vs[:],
                                         func=mybir.ActivationFunctionType.Sqrt,
                                         bias=epst[:, 0:1], scale=1.0 / 128)
                    nc.vector.reciprocal(out=vs[:], in_=vs[:])
                    nc.vector.tensor_scalar(out=xcn[:], in0=xcn[:], scalar1=vs[:, 0:1],
                                            scalar2=None, op0=mybir.AluOpType.mult)
                    nc.vector.tensor_tensor(out=xcn[:], in0=xcn[:], in1=gt[:],
                                            op=mybir.AluOpType.mult)
                    nc.vector.tensor_tensor(out=xcn[:], in0=xcn[:], in1=bt[:],
                                            op=mybir.AluOpType.add)
                    nc.sync.dma_start(out=dest[base:base + 128, :], in_=xcn[:])

            if l == 0:
                nc.gpsimd.collective_compute(
                    "AllGather", mybir.AluOpType.bypass,
                    replica_groups=[list(range(NCORE))],
                    ins=[hloc[1].ap().opt()], outs=[xfull[1].ap().opt()])

    nc.compile()
    return nc


def kernel(**inputs):
    names = np.asarray(inputs['names'])
    src_idx = np.asarray(inputs['src_idx'])
    dst_idx = np.asarray(inputs['dst_idx'])
    emb = np.asarray(inputs['node_emb'], np.float32)
    N = names.shape[1]
    V = emb.shape[0]
    P = pack(names, src_idx, dst_idx, N)
    P['V'] = V
    fw = fold_weights(inputs)
    nc = build_nc(P, fw['alphas'])

    ntile, nslot, nch = P['ntile'], P['nslot'], P['nch']
    iota = np.tile(np.arange(128, dtype=np.float32), (128, 1))
    ident = np.eye(128, dtype=np.float32)
    onesr = np.ones((1, 128), np.float32)
    com = dict(
        emb=emb,
        kw=fw['KW'].reshape(L * T * 128, D), wmsg=fw['WMSG'].reshape(L * R * 128, D),
        msgb=np.repeat(fw['MSGB'].reshape(L * R, 1, D), 128, 1).reshape(L * R * 128, D),
        w2=fw['W2'].reshape(L * R * 128, 132),
        b2=fw['B2'].reshape(L * R, 132),
        adw=np.ascontiguousarray(inputs['adapt_w'], np.float32).reshape(T * 128, D),
        adb=np.asarray(inputs['adapt_b'], np.float32),
        aw=np.ascontiguousarray(inputs['a_w'], np.float32).reshape(L * T * 128, D),
        ab=np.asarray(inputs['a_b'], np.float32).reshape(L * T, D),
        lng=np.repeat(np.asarray(inputs['ln_g'], np.float32).reshape(L * T, 1, D), 128, 1).reshape(L * T * 128, D),
        lnb=np.repeat(np.asarray(inputs['ln_b'], np.float32).reshape(L * T, 1, D), 128, 1).reshape(L * T * 128, D),
        iota=iota, ident=ident, ones=onesr,
    )
    in_maps = []
    for c in range(NCORE):
        m = dict(com)
        m['embidx'] = P['embidxT'][c]
        m['srcrow'] = P['srcrowT'][:, c].reshape(R * 128, nch)
        m['qtidx'] = P['qtidxT'][:, c].reshape(R * 128, nch)
        m['dstoff'] = P['dstoffT'][:, c].reshape(R * 128, nch)
        m['invt'] = P['invT'][c].reshape(T * 128, ntile)
        in_maps.append(m)
    import os
    trace = os.environ.get("KBENCH_TRACE", "0") == "1"
    res = run_bass_kernel_spmd(nc, in_maps, core_ids=list(range(NCORE)), trace=trace)
    if trace and res.exec_time_ns:
        print(f"HW exec time: {res.exec_time_ns} ns")
    outs = [res.results[c]["outloc"] for c in range(NCORE)]
    return unpack_output(P, outs, N)

